# revision 11
# baseline (speedup 1.0000x reference)
"""Trainium2 Bass kernel for a 2-layer GCN encoder (gnn_message_passing).

Contract: kernel(**inputs) takes FULL unsharded inputs (as produced by the
reference setup_inputs) and returns the FULL output tuple
(dense_x, enc, valid_mask, (h_1, h_2)) matching reference().

Strategy (8 NeuronCores, node-range sharding):
  - GCN conv is linear, so aggregate RAW node features per edge and apply the
    (folded) weight matrices after aggregation:
        h_next = relu(agg_in @ (gin_W @ fcW1) + agg_out @ (gout_W @ fcW2) + b_fold)
    where agg_in[c] = sum_e norm_e * x[src_e] over edges with target c.
  - Per-edge work on device: GPSIMD dma_gather fetches fp16 feature rows
    (embedding table for layer 1, h1 for layer 2); DVE builds a one-hot x norm
    selection matrix per 128-message block (tensor_scalar is_equal*mult vs a
    constant iota tile); TensorE matmuls accumulate feature-major aggregates
    [feat, 128 targets] in PSUM per 128-target window.
  - Cores own node ranges of 6250; each processes edges whose aggregation
    target is local. Messages are sorted by (table-half, target window) with
    per-(core,window) runs padded to 128 so the SPMD program structure is
    shared across cores (pad slots have tgt=-1 -> zero selection column).
  - Two launches, no collectives: launch 1 computes h1 (feature-major) per
    core; host concatenates to the full h1 table; launch 2 runs layer 2 and
    the pooling/head/output tail.
"""

import numpy as np
from contextlib import ExitStack

from concourse import bacc, bass, mybir, tile
from concourse.bass_utils import run_bass_kernel_spmd

# set by test harnesses: when True, request an NTFF profile from the runs
TRACE = False
LAST_STATS = {}

# problem constants (hardcoded per harness contract)
N = 50000
E = 800000
G = 16
H = 128
VOCAB = 32000
NPG = N // G            # 3125
M = 8                   # cores
NLOC = N // M           # 6250
P = 128
NWIN = (NLOC + P - 1) // P   # 49
NPAD = NWIN * P              # 6272
HALF = N // 2                # 25000 (h1 table split for int16 gather indices)
CHUNK_SLOTS = 8192

F16 = mybir.dt.float16
F32 = mybir.dt.float32
I16 = mybir.dt.int16
AF = mybir.ActivationFunctionType
ALU = mybir.AluOpType


# ----------------------------------------------------------------------------
# host-side helpers
# ----------------------------------------------------------------------------

def _wrap_idx16(g):
    """slot i -> [partition i%16 (replicated x8), col i//16], int16."""
    cols = len(g) // 16
    a = g.reshape(cols, 16).T.astype(np.int16)
    return np.ascontiguousarray(np.tile(a, (8, 1)))


def _wrap128(v):
    """slot i -> [partition i%128, col i//128]."""
    return np.ascontiguousarray(v.reshape(-1, P).T)


class Stream:
    """Host-built message stream for one (layer, direction).

    Shared across cores: groups (emit order), chunks, total slots.
    Per core: gidx/tgt/nrm arrays.
    """

    def __init__(self, tgt_glob, tok, tab, nrm, n_tables):
        # per-core raw selections
        per_core = []
        for k in range(M):
            sel = (tgt_glob // NLOC) == k
            t_loc = tgt_glob[sel] - k * NLOC
            w = t_loc // P
            tb = tab[sel]
            # snake order over tables within each window so same-table runs
            # merge across adjacent windows (fewer, larger gathers)
            tab_pos = np.where(w % 2 == 0, tb, (n_tables - 1) - tb)
            key = w * n_tables + tab_pos
            o = np.argsort(key, kind="stable")
            per_core.append(
                dict(
                    key=key[o],
                    tok=tok[sel][o],
                    tl=(t_loc % P)[o].astype(np.float32),
                    nr=nrm[sel][o].astype(np.float32),
                )
            )

        n_groups = NWIN * n_tables
        counts = np.zeros((M, n_groups), np.int64)
        for k in range(M):
            counts[k] = np.bincount(per_core[k]["key"], minlength=n_groups)
        blocks_per_group = (np.ceil(counts / P).max(axis=0)).astype(np.int64)

        # group g = (window, tab_pos); table id for gather region
        def group_table(g):
            w, tp = divmod(g, n_tables)
            return tp if w % 2 == 0 else (n_tables - 1) - tp

        def group_window(g):
            return g // n_tables

        self.n_tables = n_tables
        self.groups = []  # (table, window, nblocks, slot0)
        slot0 = 0
        for g in range(n_groups):
            nb = int(blocks_per_group[g])
            if nb == 0:
                continue
            self.groups.append((group_table(g), group_window(g), nb, slot0))
            slot0 += nb * P
        self.S = slot0
        assert self.S % P == 0

        # per-core slot arrays
        self.gidx = []
        self.tgt = []
        self.nrm = []
        for k in range(M):
            gtok = np.zeros(self.S, np.int64)
            tl = np.full(self.S, -1.0, np.float32)
            nr = np.zeros(self.S, np.float32)
            pc = per_core[k]
            starts = np.concatenate([[0], np.cumsum(counts[k])])
            gi = 0
            for g in range(n_groups):
                nb = int(blocks_per_group[g])
                if nb == 0:
                    continue
                a, b = starts[g], starts[g + 1]
                cnt = b - a
                s0 = self.groups[gi][3]
                gi += 1
                gtok[s0 : s0 + cnt] = pc["tok"][a:b]
                tl[s0 : s0 + cnt] = pc["tl"][a:b]
                nr[s0 : s0 + cnt] = pc["nr"][a:b]
            assert gtok.max(initial=0) < 32768
            self.gidx.append(_wrap_idx16(gtok))
            self.tgt.append(_wrap128(tl))
            self.nrm.append(_wrap128(nr))

        # chunks: maximal same-table slot runs, capped at CHUNK_SLOTS
        self.chunks = []  # (table, slot0, nslots)
        run_tab, run_s0, run_len = None, 0, 0
        for (tb, w, nb, s0) in self.groups:
            if tb == run_tab:
                run_len += nb * P
            else:
                if run_tab is not None:
                    self._push_chunks(run_tab, run_s0, run_len)
                run_tab, run_s0, run_len = tb, s0, nb * P
        if run_tab is not None:
            self._push_chunks(run_tab, run_s0, run_len)

        # block -> (window, first_of_window, last_of_window)
        self.blocks = []
        win_blocks = {}
        for (tb, w, nb, s0) in self.groups:
            win_blocks.setdefault(w, 0)
            win_blocks[w] += nb
        seen = {}
        for (tb, w, nb, s0) in self.groups:
            for i in range(nb):
                seen.setdefault(w, 0)
                seen[w] += 1
                self.blocks.append((w, seen[w] == 1, seen[w] == win_blocks[w]))

    def _push_chunks(self, tb, s0, ln):
        off = 0
        while off < ln:
            n = min(CHUNK_SLOTS, ln - off)
            self.chunks.append((tb, s0 + off, n))
            off += n


def _build_layer_streams(row, col, norm_in, norm_out, tok_of_src, tab_of_src, n_tables):
    """Returns (stream_in, stream_out) for one conv layer.

    a_in aggregates at col (sources row); a_out aggregates at row (sources col).
    """
    s_in = Stream(col, tok_of_src(row), tab_of_src(row), norm_in, n_tables)
    s_out = Stream(row, tok_of_src(col), tab_of_src(col), norm_out, n_tables)
    return s_in, s_out


# ----------------------------------------------------------------------------
# device program emission
# ----------------------------------------------------------------------------

def _emit_stream(nc, tc, ctx, pools, stream, table_aps, meta_tiles, acc_tile):
    """Emit gathers + selection builds + matmul accumulation + window flushes.

    acc_tile: SBUF fp16 [H, NPAD]; window w's aggregate lands in cols w*128..+128.
    """
    gidx_t, tgt_t, nrm_t, iota_t = meta_tiles
    msg_pool, sel_pool, psum_pool = pools

    # map chunks to covering blocks
    blk_of_slot = lambda s: s // P
    psum_cur = {}
    for (tb, s0, nslots) in stream.chunks:
        nb = nslots // P
        buf = msg_pool.tile([P, nb, H], F16, tag="msgbuf")
        nc.gpsimd.dma_gather(
            buf[:],
            table_aps[tb],
            gidx_t[:, s0 // 16 : (s0 + nslots) // 16],
            num_idxs=nslots,
            num_idxs_reg=nslots,
            elem_size=H,
            single_packet=False,
        )
        for i in range(nb):
            b = blk_of_slot(s0) + i
            w, first, last = stream.blocks[b]
            sel = sel_pool.tile([P, P], F16, tag="sel")
            nc.vector.tensor_scalar(
                sel[:],
                iota_t[:],
                tgt_t[:, b : b + 1],
                nrm_t[:, b : b + 1],
                op0=ALU.is_equal,
                op1=ALU.mult,
            )
            if first:
                wt = psum_pool.tile([H, P], F32, tag="aggwin")
                psum_cur[w] = wt
            nc.tensor.matmul(
                psum_cur[w][:],
                lhsT=buf[:, i, :],
                rhs=sel[:],
                start=first,
                stop=last,
            )
            if last:
                # flush: PSUM fp32 -> fp16 acc slice
                nc.scalar.activation(
                    acc_tile[:, w * P : (w + 1) * P],
                    psum_cur[w][:],
                    AF.Copy,
                )
                del psum_cur[w]


def _load_stream_meta(nc, pool, stream, t_gidx, t_tgt, t_nrm):
    gidx_t = pool.tile(list(t_gidx.shape), I16, tag="gidx")
    nc.sync.dma_start(gidx_t[:], t_gidx.ap())
    tgt_t = pool.tile(list(t_tgt.shape), F32, tag="tgt")
    nc.sync.dma_start(tgt_t[:], t_tgt.ap())
    nrm_t = pool.tile(list(t_nrm.shape), F32, tag="nrm")
    nc.sync.dma_start(nrm_t[:], t_nrm.ap())
    return gidx_t, tgt_t, nrm_t


def _build_p1(s_in, s_out):
    """Launch 1: layer-1 aggregation + folded fc + relu -> h1T per core."""
    nc = bacc.Bacc("TRN2", target_bir_lowering=False, debug=False)
    t_emb = nc.dram_tensor("emb16", [VOCAB, H], F16, kind="ExternalInput")
    t_io = nc.dram_tensor("iota", [P, P], F16, kind="ExternalInput")
    t_wi = nc.dram_tensor("wfold_in", [H, H], F16, kind="ExternalInput")
    t_wo = nc.dram_tensor("wfold_out", [H, H], F16, kind="ExternalInput")
    t_bf = nc.dram_tensor("bfold", [H, 1], F32, kind="ExternalInput")
    tens = {}
    for nm, st in (("si", s_in), ("so", s_out)):
        tens[nm] = (
            nc.dram_tensor(f"{nm}_gidx", list(st.gidx[0].shape), I16, kind="ExternalInput"),
            nc.dram_tensor(f"{nm}_tgt", list(st.tgt[0].shape), F32, kind="ExternalInput"),
            nc.dram_tensor(f"{nm}_nrm", list(st.nrm[0].shape), F32, kind="ExternalInput"),
        )
    t_h1T = nc.dram_tensor("h1T", [H, NPAD], F16, kind="ExternalOutput")

    with tile.TileContext(nc) as tc, ExitStack() as ctx:
        const_pool = ctx.enter_context(tc.tile_pool(name="const", bufs=1))
        meta_pool = ctx.enter_context(tc.tile_pool(name="meta", bufs=2))
        msg_pool = ctx.enter_context(tc.tile_pool(name="msg", bufs=3))
        sel_pool = ctx.enter_context(tc.tile_pool(name="sel", bufs=4))
        acc_pool = ctx.enter_context(tc.tile_pool(name="acc", bufs=1))
        agg_psum = ctx.enter_context(tc.tile_pool(name="aggps", bufs=2, space="PSUM"))
        tail_psum = ctx.enter_context(tc.tile_pool(name="tailps", bufs=2, space="PSUM"))
        out_pool = ctx.enter_context(tc.tile_pool(name="outp", bufs=1))

        iota_t = const_pool.tile([P, P], F16)
        nc.sync.dma_start(iota_t[:], t_io.ap())
        wi_t = const_pool.tile([H, H], F16)
        nc.sync.dma_start(wi_t[:], t_wi.ap())
        wo_t = const_pool.tile([H, H], F16)
        nc.sync.dma_start(wo_t[:], t_wo.ap())
        bf_t = const_pool.tile([H, 1], F32)
        nc.sync.dma_start(bf_t[:], t_bf.ap())

        acc_in = acc_pool.tile([H, NPAD], F16, tag="acc_in")
        acc_out = acc_pool.tile([H, NPAD], F16, tag="acc_out")
        h1T_t = out_pool.tile([H, NPAD], F16)

        table_aps = [t_emb.ap()]
        for st, nm, acc in ((s_in, "si", acc_in), (s_out, "so", acc_out)):
            mt = _load_stream_meta(nc, meta_pool, st, *tens[nm])
            _emit_stream(
                nc, tc, ctx,
                (msg_pool, sel_pool, agg_psum),
                st, table_aps, (*mt, iota_t), acc,
            )

        for w in range(NWIN):
            cs = slice(w * P, (w + 1) * P)
            hp = tail_psum.tile([H, P], F32, tag="tailps")
            nc.tensor.matmul(hp[:], lhsT=wi_t[:], rhs=acc_in[:, cs], start=True, stop=False)
            nc.tensor.matmul(hp[:], lhsT=wo_t[:], rhs=acc_out[:, cs], start=False, stop=True)
            nc.scalar.activation(h1T_t[:, cs], hp[:], AF.Relu, bias=bf_t[:])
        nc.sync.dma_start(t_h1T.ap(), h1T_t[:])

    nc.compile()
    return nc


def _build_p2(s_in, s_out):
    """Launch 2: layer-2 aggregation + fc + pooling/heads/enc/dense outputs."""
    nc = bacc.Bacc("TRN2", target_bir_lowering=False, debug=False)
    t_h1a = nc.dram_tensor("h1a", [HALF, H], F16, kind="ExternalInput")
    t_h1b = nc.dram_tensor("h1b", [HALF, H], F16, kind="ExternalInput")
    t_io = nc.dram_tensor("iota", [P, P], F16, kind="ExternalInput")
    t_id = nc.dram_tensor("ident", [P, P], F16, kind="ExternalInput")
    t_wi = nc.dram_tensor("wfold_in", [H, H], F16, kind="ExternalInput")
    t_wo = nc.dram_tensor("wfold_out", [H, H], F16, kind="ExternalInput")
    t_bf = nc.dram_tensor("bfold", [H, 1], F32, kind="ExternalInput")
    t_w5 = nc.dram_tensor("w5", [H, H], F16, kind="ExternalInput")
    t_b5 = nc.dram_tensor("b5", [H, 1], F32, kind="ExternalInput")
    t_wh = nc.dram_tensor("wheads", [H, 4 * H], F16, kind="ExternalInput")
    t_bh = nc.dram_tensor("bheads", [H, 4], F32, kind="ExternalInput")
    t_pm = nc.dram_tensor("pmask", [P, NWIN * 2], F32, kind="ExternalInput")
    tens = {}
    for nm, st in (("si", s_in), ("so", s_out)):
        tens[nm] = (
            nc.dram_tensor(f"{nm}_gidx", list(st.gidx[0].shape), I16, kind="ExternalInput"),
            nc.dram_tensor(f"{nm}_tgt", list(st.tgt[0].shape), F32, kind="ExternalInput"),
            nc.dram_tensor(f"{nm}_nrm", list(st.nrm[0].shape), F32, kind="ExternalInput"),
        )
    t_dense = nc.dram_tensor("dense", [P, NWIN * H], F32, kind="ExternalOutput")
    t_enc = nc.dram_tensor("enc", [P, NWIN * H], F32, kind="ExternalOutput")
    t_heads = nc.dram_tensor("heads", [H, 8], F32, kind="ExternalOutput")

    with tile.TileContext(nc) as tc, ExitStack() as ctx:
        const_pool = ctx.enter_context(tc.tile_pool(name="const", bufs=1))
        meta_pool = ctx.enter_context(tc.tile_pool(name="meta", bufs=2))
        msg_pool = ctx.enter_context(tc.tile_pool(name="msg", bufs=3))
        sel_pool = ctx.enter_context(tc.tile_pool(name="sel", bufs=4))
        acc_pool = ctx.enter_context(tc.tile_pool(name="acc", bufs=1))
        agg_psum = ctx.enter_context(tc.tile_pool(name="aggps", bufs=2, space="PSUM"))
        tail_psum = ctx.enter_context(tc.tile_pool(name="tailps", bufs=4, space="PSUM"))
        ge_psum = ctx.enter_context(tc.tile_pool(name="geps", bufs=1, space="PSUM"))
        tail_pool = ctx.enter_context(tc.tile_pool(name="tails", bufs=4))
        big_pool = ctx.enter_context(tc.tile_pool(name="big", bufs=1))

        iota_t = const_pool.tile([P, P], F16)
        nc.sync.dma_start(iota_t[:], t_io.ap())
        ident_t = const_pool.tile([P, P], F16)
        nc.sync.dma_start(ident_t[:], t_id.ap())
        wi_t = const_pool.tile([H, H], F16)
        nc.sync.dma_start(wi_t[:], t_wi.ap())
        wo_t = const_pool.tile([H, H], F16)
        nc.sync.dma_start(wo_t[:], t_wo.ap())
        bf_t = const_pool.tile([H, 1], F32)
        nc.sync.dma_start(bf_t[:], t_bf.ap())
        w5_t = const_pool.tile([H, H], F16)
        nc.sync.dma_start(w5_t[:], t_w5.ap())
        b5_t = const_pool.tile([H, 1], F32)
        nc.sync.dma_start(b5_t[:], t_b5.ap())
        wh_t = const_pool.tile([H, 4 * H], F16)
        nc.sync.dma_start(wh_t[:], t_wh.ap())
        bh_t = const_pool.tile([H, 4], F32)
        nc.sync.dma_start(bh_t[:], t_bh.ap())
        pm_t = const_pool.tile([P, NWIN * 2], F32)
        nc.sync.dma_start(pm_t[:], t_pm.ap())

        acc_in = acc_pool.tile([H, NPAD], F16, tag="acc_in")
        acc_out = acc_pool.tile([H, NPAD], F16, tag="acc_out")
        h2T_all = acc_pool.tile([H, NPAD], F16, tag="h2T")
        dense_sb = big_pool.tile([P, NWIN * H], F32, tag="dense")
        enc_sb = big_pool.tile([P, NWIN * H], F32, tag="encsb")

        table_aps = [t_h1a.ap(), t_h1b.ap()]
        for st, nm, acc in ((s_in, "si", acc_in), (s_out, "so", acc_out)):
            mt = _load_stream_meta(nc, meta_pool, st, *tens[nm])
            _emit_stream(
                nc, tc, ctx,
                (msg_pool, sel_pool, agg_psum),
                st, table_aps, (*mt, iota_t), acc,
            )

        gep = ge_psum.tile([H, 2], F32)
        for w in range(NWIN):
            cs = slice(w * P, (w + 1) * P)
            fs = slice(w * H, (w + 1) * H)
            hp = tail_psum.tile([H, P], F32, tag="tailps")
            nc.tensor.matmul(hp[:], lhsT=wi_t[:], rhs=acc_in[:, cs], start=True, stop=False)
            nc.tensor.matmul(hp[:], lhsT=wo_t[:], rhs=acc_out[:, cs], start=False, stop=True)
            nc.scalar.activation(h2T_all[:, cs], hp[:], AF.Relu, bias=bf_t[:])

            # dense (node-major fp32) via PE transpose
            tp = tail_psum.tile([P, H], F32, tag="tailps")
            nc.tensor.matmul(tp[:], lhsT=h2T_all[:, cs], rhs=ident_t[:], start=True, stop=True)
            nc.scalar.activation(dense_sb[:, fs], tp[:], AF.Copy)

            # encT = relu(w5^T @ h2T + b5) then transpose to node-major fp32
            ep = tail_psum.tile([H, P], F32, tag="tailps")
            nc.tensor.matmul(ep[:], lhsT=w5_t[:], rhs=h2T_all[:, cs], start=True, stop=True)
            encT = tail_pool.tile([H, P], F16, tag="encT")
            nc.scalar.activation(encT[:], ep[:], AF.Relu, bias=b5_t[:])
            tp2 = tail_psum.tile([P, H], F32, tag="tailps")
            nc.tensor.matmul(tp2[:], lhsT=encT[:], rhs=ident_t[:], start=True, stop=True)
            nc.scalar.activation(enc_sb[:, fs], tp2[:], AF.Copy)

            # pooled geT accumulation: lhsT = dense window (node-major fp32)
            nc.tensor.matmul(
                gep[:],
                lhsT=dense_sb[:, fs],
                rhs=pm_t[:, 2 * w : 2 * w + 2],
                start=(w == 0),
                stop=(w == NWIN - 1),
            )

        geT = tail_pool.tile([H, 2], F16)
        nc.scalar.activation(geT[:], gep[:], AF.Copy)
        heads_sb = tail_pool.tile([H, 8], F32)
        for j in range(4):
            hps = tail_psum.tile([H, 2], F32, tag="tailps")
            nc.tensor.matmul(
                hps[:], lhsT=wh_t[:, j * H : (j + 1) * H], rhs=geT[:], start=True, stop=True
            )
            nc.scalar.activation(
                heads_sb[:, 2 * j : 2 * j + 2], hps[:], AF.Identity, bias=bh_t[:, j : j + 1]
            )

        nc.sync.dma_start(t_dense.ap(), dense_sb[:])
        nc.sync.dma_start(t_enc.ap(), enc_sb[:])
        nc.sync.dma_start(t_heads.ap(), heads_sb[:])

    nc.compile()
    return nc


# ----------------------------------------------------------------------------
# host fallback (exact reference math in numpy) for unexpected input structure
# ----------------------------------------------------------------------------

def _host_reference(src_x, edge_index, batch, params):
    n = src_x.shape[0]
    loop = np.arange(n)
    row = np.concatenate([edge_index[0], loop])
    col = np.concatenate([edge_index[1], loop])

    def conv(x, W, b, deg_idx, gather_idx, aggr_idx):
        deg = np.bincount(deg_idx, minlength=n).astype(np.float32)
        dinv = (1.0 / np.sqrt(deg)).astype(np.float32)
        norm = dinv[row] * dinv[col]
        msg = (x @ W)[gather_idx] * norm[:, None]
        out = np.zeros((n, W.shape[1]), np.float32)
        np.add.at(out, aggr_idx, msg)
        return out + b

    h = params["emb"][src_x].astype(np.float32)
    for l in range(2):
        p = params[f"l{l}"]
        a_in = conv(h, p["gin_W"], p["gin_b"], col, row, col)
        a_out = conv(h, p["gout_W"], p["gout_b"], row, col, row)
        h = np.maximum(np.concatenate([a_in, a_out], 1) @ p["fc_W"] + p["fc_b"], 0.0)

    cnt = np.bincount(batch, minlength=G).astype(np.float32)
    ge = np.zeros((G, H), np.float32)
    np.add.at(ge, batch, h)
    ge = ge / cnt[:, None]

    def head(name):
        return (ge @ params[name + "_W"] + params[name + "_b"]).reshape(1, G, H)

    h_1 = np.concatenate([head("w1"), head("w2")], 0)
    h_2 = np.concatenate([head("w3"), head("w4")], 0)

    cnt_i = np.bincount(batch, minlength=G)
    starts = np.concatenate([[0], np.cumsum(cnt_i)[:-1]])
    pos = np.arange(n) - starts[batch]
    dense_x = np.zeros((G, NPG, H), np.float32)
    valid_mask = np.zeros((G, NPG), bool)
    ok = pos < NPG
    dense_x[batch[ok], pos[ok]] = h[ok]
    valid_mask[batch[ok], pos[ok]] = True
    enc = np.maximum(dense_x.reshape(-1, H) @ params["w5_W"] + params["w5_b"], 0.0)
    return dense_x, enc, valid_mask, (h_1, h_2)


# ----------------------------------------------------------------------------
# entry point
# ----------------------------------------------------------------------------

def _np_params(params):
    out = {}
    for k, v in params.items():
        out[k] = _np_params(v) if isinstance(v, dict) else np.asarray(v)
    return out


def kernel(src_x, edge_index, batch, params):
    src_x = np.asarray(src_x).astype(np.int64)
    edge_index = np.asarray(edge_index).astype(np.int64)
    batch = np.asarray(batch).astype(np.int64)
    params = _np_params(params)

    expected_batch = np.repeat(np.arange(G), NPG)
    if (
        src_x.shape != (N,)
        or edge_index.shape != (2, E)
        or batch.shape != (N,)
        or not np.array_equal(batch, expected_batch)
    ):
        return _host_reference(src_x, edge_index, batch, params)

    # ---- host index preprocessing ----
    loop = np.arange(N)
    row = np.concatenate([edge_index[0], loop])
    col = np.concatenate([edge_index[1], loop])
    deg_in = np.bincount(col, minlength=N).astype(np.float32)
    deg_out = np.bincount(row, minlength=N).astype(np.float32)
    dinv_in = (1.0 / np.sqrt(deg_in)).astype(np.float32)
    dinv_out = (1.0 / np.sqrt(deg_out)).astype(np.float32)
    norm_in = dinv_in[row] * dinv_in[col]
    norm_out = dinv_out[row] * dinv_out[col]

    # layer 1: gather directly from the embedding table (token = src_x[node])
    s1_in, s1_out = _build_layer_streams(
        row, col, norm_in, norm_out,
        tok_of_src=lambda s: src_x[s],
        tab_of_src=lambda s: np.zeros_like(s),
        n_tables=1,
    )
    # layer 2: gather from h1 split into two halves
    s2_in, s2_out = _build_layer_streams(
        row, col, norm_in, norm_out,
        tok_of_src=lambda s: s % HALF,
        tab_of_src=lambda s: s // HALF,
        n_tables=2,
    )

    # folded weights
    fc0, fc1 = params["l0"]["fc_W"], params["l1"]["fc_W"]
    wf1_in = (params["l0"]["gin_W"] @ fc0[:H]).astype(np.float16)
    wf1_out = (params["l0"]["gout_W"] @ fc0[H:]).astype(np.float16)
    bf1 = (
        params["l0"]["gin_b"] @ fc0[:H]
        + params["l0"]["gout_b"] @ fc0[H:]
        + params["l0"]["fc_b"]
    ).astype(np.float32)[:, None]
    wf2_in = (params["l1"]["gin_W"] @ fc1[:H]).astype(np.float16)
    wf2_out = (params["l1"]["gout_W"] @ fc1[H:]).astype(np.float16)
    bf2 = (
        params["l1"]["gin_b"] @ fc1[:H]
        + params["l1"]["gout_b"] @ fc1[H:]
        + params["l1"]["fc_b"]
    ).astype(np.float32)[:, None]

    emb16 = params["emb"].astype(np.float16)
    iota_np = np.ascontiguousarray(np.tile(np.arange(P, dtype=np.float16), (P, 1)))
    ident_np = np.eye(P, dtype=np.float16)

    # pooling mask (node-in-window, 2 local graphs per core), entries 1/NPG
    pmask = np.zeros((P, NWIN * 2), np.float32)
    for w in range(NWIN):
        for pp in range(P):
            nl = w * P + pp
            if nl < NLOC:
                pmask[pp, 2 * w + (nl // NPG)] = 1.0 / NPG

    wheads = np.concatenate(
        [params[f"w{j}_W"].astype(np.float16) for j in (1, 2, 3, 4)], axis=1
    )
    bheads = np.stack(
        [params[f"w{j}_b"].astype(np.float32) for j in (1, 2, 3, 4)], axis=1
    )

    # ---- launch 1 ----
    nc1 = _build_p1(s1_in, s1_out)
    in_maps1 = []
    for k in range(M):
        in_maps1.append(
            {
                "emb16": emb16,
                "iota": iota_np,
                "wfold_in": wf1_in,
                "wfold_out": wf1_out,
                "bfold": bf1,
                "si_gidx": s1_in.gidx[k],
                "si_tgt": s1_in.tgt[k],
                "si_nrm": s1_in.nrm[k],
                "so_gidx": s1_out.gidx[k],
                "so_tgt": s1_out.tgt[k],
                "so_nrm": s1_out.nrm[k],
            }
        )
    import time as _time

    _t0 = _time.time()
    res1 = run_bass_kernel_spmd(nc1, in_maps1, list(range(M)), trace=TRACE)
    LAST_STATS["p1_wall_s"] = _time.time() - _t0
    LAST_STATS["p1_exec_ns"] = getattr(res1, "exec_time_ns", None)
    h1 = np.empty((N, H), np.float16)
    for k in range(M):
        h1[k * NLOC : (k + 1) * NLOC] = res1.results[k]["h1T"][:, :NLOC].T
    h1a, h1b = np.ascontiguousarray(h1[:HALF]), np.ascontiguousarray(h1[HALF:])

    # ---- launch 2 ----
    nc2 = _build_p2(s2_in, s2_out)
    in_maps2 = []
    for k in range(M):
        in_maps2.append(
            {
                "h1a": h1a,
                "h1b": h1b,
                "iota": iota_np,
                "ident": ident_np,
                "wfold_in": wf2_in,
                "wfold_out": wf2_out,
                "bfold": bf2,
                "w5": params["w5_W"].astype(np.float16),
                "b5": params["w5_b"].astype(np.float32)[:, None],
                "wheads": wheads,
                "bheads": bheads,
                "pmask": pmask,
                "si_gidx": s2_in.gidx[k],
                "si_tgt": s2_in.tgt[k],
                "si_nrm": s2_in.nrm[k],
                "so_gidx": s2_out.gidx[k],
                "so_tgt": s2_out.tgt[k],
                "so_nrm": s2_out.nrm[k],
            }
        )
    _t0 = _time.time()
    res2 = run_bass_kernel_spmd(nc2, in_maps2, list(range(M)), trace=TRACE)
    LAST_STATS["p2_wall_s"] = _time.time() - _t0
    LAST_STATS["p2_exec_ns"] = getattr(res2, "exec_time_ns", None)

    dense_flat = np.empty((N, H), np.float32)
    enc = np.empty((N, H), np.float32)
    h_1 = np.empty((2, G, H), np.float32)
    h_2 = np.empty((2, G, H), np.float32)
    for k in range(M):
        r = res2.results[k]
        d = r["dense"].reshape(P, NWIN, H).transpose(1, 0, 2).reshape(NPAD, H)
        e = r["enc"].reshape(P, NWIN, H).transpose(1, 0, 2).reshape(NPAD, H)
        dense_flat[k * NLOC : (k + 1) * NLOC] = d[:NLOC]
        enc[k * NLOC : (k + 1) * NLOC] = e[:NLOC]
        hd = r["heads"]  # [H, 8]
        for j in range(4):
            tgt = h_1 if j < 2 else h_2
            tgt[j % 2, 2 * k] = hd[:, 2 * j]
            tgt[j % 2, 2 * k + 1] = hd[:, 2 * j + 1]

    dense_x = dense_flat.reshape(G, NPG, H)
    valid_mask = np.ones((G, NPG), bool)
    return dense_x, enc, valid_mask, (h_1, h_2)
